# revision 11
# baseline (speedup 1.0000x reference)
"""Trainium2 Bass kernel for nn_MixedAttention (ConvBERT-style mixed attention).

Sharding: data-parallel over (batch=4) x (seq halves=2) = 8 cores.
Each core computes output rows [j*1024, (j+1)*1024) of batch b, core = 2*b + j.
k/v are computed redundantly on both cores of a batch pair (no collectives);
attention keys are ordered [local chunk, other chunk] per core (softmax is
key-order invariant as long as kT and v agree).

Per-core layout strategy (all SBUF tiles [<=128 partitions, free]):
  xT (hidden on partitions, seq on free) drives every projection matmul.
  q,k,co,conv_attn live transposed [a, s]; v lives natural [s, a] (for ctx
  matmul). Attention is computed as S.T = kT.T @ qT tiles [sk=128, sq], exp'd
  on ACT (scale=1/8 folded in), and ctx.T accumulated via lhsT=[v_h | ones] so
  the softmax denominator falls out as row 64; normalization happens after a
  PE transpose back to [s, d] layout.
  Conv branch: depthwise conv as 9 shifted per-partition-scalar FMAs split
  across Pool (seq half 1) and DVE (seq half 0), fp16 accumulators; pointwise
  matmul (fp16) fused with the conv_attn multiply on evacuation; dynamic span
  kernel softmax'd per head via a selector matmul; span weights broadcast
  across head dims by SBUF->SBUF DMA; windowed einsum as 9 multiply-adds on
  DVE overlapped under the attention phase.

Emission order is tuned for the in-order per-engine queues: PE runs
q, co, k, v, pw, ckl, den, then attention heads with the conv transposes
interleaved; DVE runs bias evacs with the depthwise halves slotted into its
idle windows, then the windowed einsum, then per-head normalization; Pool
runs the other depthwise half and ctx PSUM evacuations. Outputs stream out
per head.
"""

import sys

for _p in ("/opt/trn_rl_repo",):
    if _p not in sys.path:
        sys.path.insert(0, _p)

import numpy as np
import ml_dtypes

HIDDEN = 768
N_HEADS = 6
HEAD_DIM = 64
ALL_HEAD = 384
K = 9
B, S = 4, 2048
CHUNK = 1024          # seq rows per core
N_CORES = 8
BF16 = ml_dtypes.bfloat16
FP16 = np.float16

_COMPILED = {}


def _build_program():
    import concourse.bass as bass
    import concourse.mybir as mybir
    import concourse.tile as tile
    from concourse import bacc
    from concourse.masks import make_identity
    from contextlib import ExitStack

    dt = mybir.dt
    Alu = mybir.AluOpType
    Act = mybir.ActivationFunctionType

    nc = bacc.Bacc("TRN2", target_bir_lowering=False, debug=False)

    # ---------------- DRAM I/O (host pre-laid in SBUF layout) ----------------
    def din(name, shape, dtype=dt.bfloat16):
        return nc.dram_tensor(name, list(shape), dtype, kind="ExternalInput").ap()

    x_loc = din("x_loc", [128, 6 * 1032])             # xT chunk+-4 (padded)
    x_oth = din("x_oth", [128, 6 * 1024])             # xT other chunk
    wq = din("wq", [128, 6 * ALL_HEAD])
    wk = din("wk", [128, 6 * ALL_HEAD])
    wv = din("wv", [128, 6 * ALL_HEAD])
    wco = din("wco", [128, 6 * ALL_HEAD])
    wpw = din("wpw", [128, 6 * ALL_HEAD], dt.float16)
    wck = din("wck", [128, 3 * 54])
    dwsc = din("dwsc", [128, 6 * K], dt.float32)      # depthwise taps
    sel = din("sel", [54, 6])                          # head-sum selector
    bvrow = din("bvrow", [1, ALL_HEAD])
    comask = din("comask", [1, 1032])
    bq = din("bq", [128, 3], dt.float32)
    bk = din("bk", [128, 3], dt.float32)
    convb = din("convb", [128, 3], dt.float32)
    bco = din("bco", [128, 3], dt.float32)
    bck = din("bck", [54, 1], dt.float32)

    out = nc.dram_tensor("out", [128, 8 * 768], dt.float32, kind="ExternalOutput").ap()
    pck_dram = nc.dram_tensor("pck_scratch", [54, CHUNK], dt.bfloat16).ap()

    with tile.TileContext(nc) as tc, ExitStack() as ctx:
        singles = ctx.enter_context(tc.tile_pool(name="singles", bufs=1))
        persist = ctx.enter_context(tc.tile_pool(name="persist", bufs=1))
        work = ctx.enter_context(tc.tile_pool(name="work", bufs=3))

        def load(pool, src, shape, dtype=dt.bfloat16, name=None):
            t = pool.tile(shape, dtype, name=name)
            nc.sync.dma_start(out=t, in_=src)
            return t

        # ---------------- load inputs (issue order = priority) --------------
        xlsb = singles.tile([128, 6, 1032], dt.bfloat16, name="xlsb")
        xosb = singles.tile([128, 6, 1024], dt.bfloat16, name="xosb")
        wq_sb = singles.tile([128, 6, ALL_HEAD], dt.bfloat16, name="wq_sb")
        wk_sb = singles.tile([128, 6, ALL_HEAD], dt.bfloat16, name="wk_sb")
        wv_sb = singles.tile([128, 6, ALL_HEAD], dt.bfloat16, name="wv_sb")
        wco_sb = singles.tile([128, 6, ALL_HEAD], dt.bfloat16, name="wco_sb")
        wpw_sb = singles.tile([128, 6, ALL_HEAD], dt.float16, name="wpw_sb")

        xl = x_loc.rearrange("p (h s) -> p h s", h=6)
        xo = x_oth.rearrange("p (h s) -> p h s", h=6)
        wqr = wq.rearrange("p (h a) -> p h a", h=6)
        wkr = wk.rearrange("p (h a) -> p h a", h=6)
        wvr = wv.rearrange("p (h a) -> p h a", h=6)
        wcor = wco.rearrange("p (h a) -> p h a", h=6)
        wpwr = wpw.rearrange("p (h a) -> p h a", h=6)

        for dh in range(6):
            nc.sync.dma_start(out=xlsb[:, dh], in_=xl[:, dh])
            nc.sync.dma_start(out=wq_sb[:, dh], in_=wqr[:, dh])
        bq_sb = load(singles, bq, [128, 3], dt.float32, name="bq_sb")
        dwsc_sb = load(singles, dwsc, [128, 6 * K], dt.float32, name="dwsc_sb")
        for dh in range(6):
            nc.sync.dma_start(out=wco_sb[:, dh], in_=wcor[:, dh])
        bco_sb = load(singles, bco, [128, 3], dt.float32, name="bco_sb")
        mask_sb = singles.tile([128, 1032], dt.bfloat16, name="mask_sb")
        nc.sync.dma_start(out=mask_sb, in_=comask.to_broadcast([128, 1032]))
        for dh in range(6):
            nc.sync.dma_start(out=wk_sb[:, dh], in_=wkr[:, dh])
            nc.sync.dma_start(out=xosb[:, dh], in_=xo[:, dh])
        bk_sb = load(singles, bk, [128, 3], dt.float32, name="bk_sb")
        for dh in range(6):
            nc.sync.dma_start(out=wv_sb[:, dh], in_=wvr[:, dh])
        bv_sb = load(singles, bvrow, [1, ALL_HEAD], name="bv_sb")
        for dh in range(6):
            nc.sync.dma_start(out=wpw_sb[:, dh], in_=wpwr[:, dh])
        convb_sb = load(singles, convb, [128, 3], dt.float32, name="convb_sb")
        wck_sb = load(singles, wck, [128, 3, 54], name="wck_sb")
        bck_sb = load(singles, bck, [54, 1], dt.float32, name="bck_sb")
        sel_sb = load(singles, sel, [54, 6], name="sel_sb")

        ident = singles.tile([128, 128], dt.bfloat16, name="ident")
        make_identity(nc, ident)
        ones_sb = singles.tile([1, 128], dt.bfloat16, name="ones_sb")
        nc.gpsimd.memset(ones_sb, 1.0)

        # persistent intermediates
        qT = persist.tile([128, 3, CHUNK], dt.bfloat16, name="qT")
        kT = persist.tile([128, 3, S], dt.bfloat16, name="kT")
        dwT = persist.tile([128, 6, CHUNK], dt.float16, name="dwT")
        caT = persist.tile([128, 3, CHUNK], dt.bfloat16, name="caT")
        coT = persist.tile([128, 3, 1032], dt.bfloat16, name="coT")
        vsb = persist.tile([128, 16, 6, 65], dt.bfloat16, name="vsb")
        pck = persist.tile([54, CHUNK], dt.bfloat16, name="pck")
        recipc = persist.tile([128, 8, 6], dt.float32, name="recipc")
        accT = persist.tile([128, 3, CHUNK], dt.bfloat16, name="accT")
        stg = persist.tile([128, 8, 768], dt.float32, name="stg")
        nc.gpsimd.memset(vsb[:, :, :, 64:65], 1.0)

        out_r = out.rearrange("p (st c) -> p st c", st=8)

        # depthwise conv: dwT[:, ct, s] = sum_k dw[c, k] * xloc[:, ct, s + k]
        # All chains on DVE (Pool lacks TensorScalarPtr on hardware), slotted
        # into DVE's idle windows between projection evacuations.
        def dw_chain(eng, ct, o):
            eng.tensor_scalar_mul(
                dwT[:, ct, o:o + 512], xlsb[:, ct, o:o + 512],
                dwsc_sb[:, ct * K: ct * K + 1])
            for k in range(1, K):
                eng.scalar_tensor_tensor(
                    out=dwT[:, ct, o:o + 512],
                    in0=xlsb[:, ct, k + o: k + o + 512],
                    scalar=dwsc_sb[:, ct * K + k: ct * K + k + 1],
                    in1=dwT[:, ct, o:o + 512], op0=Alu.mult, op1=Alu.add)

        # ---------------- phase B1 on PE: q, k, co, v projections -----------
        with tc.tile_pool(name="psum_b1", bufs=1, space="PSUM") as pb1:
            # q projection from local window (cols 4..1028)
            for at in range(3):
                ps = [pb1.tile([128, 512], dt.float32, tag="pj", bufs=4,
                               name=f"pq{sb}") for sb in range(2)]
                for dh in range(6):
                    for sb in range(2):
                        nc.tensor.matmul(
                            ps[sb], wq_sb[:, dh, at * 128:(at + 1) * 128],
                            xlsb[:, dh, 4 + sb * 512: 4 + (sb + 1) * 512],
                            start=(dh == 0), stop=(dh == 5))
                for sb in range(2):
                    nc.vector.tensor_scalar_add(
                        qT[:, at, sb * 512:(sb + 1) * 512], ps[sb],
                        bq_sb[:, at:at + 1])
            # DVE idle window while k matmuls run: depthwise half 0
            for ct in range(6):
                dw_chain(nc.vector, ct, 0)
            # k projection: keys [local 0:1024 | other 1024:2048], sb-paired
            for at in range(3):
                for half in range(2):
                    ps = [pb1.tile([128, 512], dt.float32, tag="pj", bufs=4,
                                   name=f"pk{sb}") for sb in range(2)]
                    for dh in range(6):
                        for sb in range(2):
                            if half == 0:
                                xs = xlsb[:, dh, 4 + sb * 512: 4 + (sb + 1) * 512]
                            else:
                                xs = xosb[:, dh, sb * 512:(sb + 1) * 512]
                            nc.tensor.matmul(
                                ps[sb], wk_sb[:, dh, at * 128:(at + 1) * 128],
                                xs, start=(dh == 0), stop=(dh == 5))
                    for sb in range(2):
                        nc.vector.tensor_scalar_add(
                            kT[:, at, half * 1024 + sb * 512:
                               half * 1024 + (sb + 1) * 512],
                            ps[sb], bk_sb[:, at:at + 1])
            # second DVE depthwise slot: half 1
            for ct in range(6):
                dw_chain(nc.vector, ct, 512)
            # co projection on chunk+-4 (1032 cols); bias + OOB mask on DVE
            for at in range(3):
                for (o, w) in ((0, 512), (512, 512), (1024, 8)):
                    pco = pb1.tile([128, 512], dt.float32, tag="pj", bufs=4,
                                   name="pco")
                    for dh in range(6):
                        nc.tensor.matmul(
                            pco[:, :w], wco_sb[:, dh, at * 128:(at + 1) * 128],
                            xlsb[:, dh, o:o + w],
                            start=(dh == 0), stop=(dh == 5))
                    nc.vector.scalar_tensor_tensor(
                        out=coT[:, at, o:o + w], in0=pco[:, :w],
                        scalar=bco_sb[:, at:at + 1], in1=mask_sb[:, o:o + w],
                        op0=Alu.add, op1=Alu.mult)
            # v projection, natural [s, a] + ones column; bias via rank-1
            # matmul. st 0..7 local rows, 8..15 other rows. Evac on DVE.
            for st in range(16):
                pv = pb1.tile([128, ALL_HEAD], dt.float32, tag="pv", bufs=3,
                              name="pv")
                for dh in range(6):
                    if st < 8:
                        xs = xlsb[:, dh, 4 + st * 128: 4 + (st + 1) * 128]
                    else:
                        xs = xosb[:, dh, (st - 8) * 128: (st - 7) * 128]
                    nc.tensor.matmul(pv, xs, wv_sb[:, dh, :],
                                     start=(dh == 0), stop=False)
                nc.tensor.matmul(pv, ones_sb, bv_sb, start=False, stop=True)
                nc.vector.tensor_copy(vsb[:, st, :, 0:64], pv.rearrange(
                    "p (h d) -> p h d", h=6))

        # ---------------- conv projections + attention PSUM pool ------------
        pa = ctx.enter_context(tc.tile_pool(name="psum_at", bufs=1,
                                            space="PSUM"))

        # pointwise conv fused with conv_attn: caT = (pw@dwT + convb) * qT
        for at in range(3):
            for sb in range(2):
                ppw = pa.tile([128, 512], dt.float32, tag="sc", bufs=2,
                              name="ppw")
                for dh in range(6):
                    nc.tensor.matmul(
                        ppw, wpw_sb[:, dh, at * 128:(at + 1) * 128],
                        dwT[:, dh, sb * 512:(sb + 1) * 512],
                        start=(dh == 0), stop=(dh == 5))
                nc.vector.scalar_tensor_tensor(
                    out=caT[:, at, sb * 512:(sb + 1) * 512], in0=ppw,
                    scalar=convb_sb[:, at:at + 1],
                    in1=qT[:, at, sb * 512:(sb + 1) * 512],
                    op0=Alu.add, op1=Alu.mult)
        # conv kernel layer -> exp -> per-head denominators -> reciprocal
        for sb in range(2):
            pck_ps = pa.tile([54, 512], dt.float32, tag="sc", bufs=2,
                             name="pck_ps")
            for at in range(3):
                nc.tensor.matmul(
                    pck_ps, wck_sb[:, at, :],
                    caT[:, at, sb * 512:(sb + 1) * 512],
                    start=(at == 0), stop=(at == 2))
            nc.scalar.activation(pck[:, sb * 512:(sb + 1) * 512], pck_ps,
                                 Act.Exp, bias=bck_sb, scale=1.0)
        for st in range(8):
            pdn = pa.tile([128, 6], dt.float32, tag="sc", bufs=2, name="pdn")
            nc.tensor.matmul(
                pdn, pck[:, st * 128:(st + 1) * 128], sel_sb,
                start=True, stop=True)
            nc.vector.reciprocal(recipc[:, st, :], pdn)

        # span-weight broadcast tiles: DRAM roundtrip, 64-way partition bcast
        nc.sync.dma_start(out=pck_dram, in_=pck)
        ckbs = {}
        for at in range(3):
            for k in range(K):
                ckb = work.tile([128, CHUNK], dt.bfloat16, tag="ckb", bufs=6,
                                name="ckb")
                for hh in range(2):
                    row = 18 * at + 9 * hh + k
                    srcap = bass.AP(
                        tensor=pck_dram.tensor,
                        offset=row * CHUNK,
                        ap=[[0, 64], [1, CHUNK]])
                    nc.sync.dma_start(out=ckb[hh * 64:(hh + 1) * 64],
                                      in_=srcap)
                ckbs[(at, k)] = ckb

        # windowed einsum on DVE: accT[:,at,:] = sum_k ckb_k * coT[:,at,k:]
        # (emitted per at-tile between attention heads, see below)
        def einsum_tile(at):
            nc.vector.tensor_mul(accT[:, at, :], ckbs[(at, 0)],
                                 coT[:, at, 0:CHUNK])
            for k in range(1, K):
                tmp = work.tile([128, CHUNK], dt.bfloat16, tag="tmp", bufs=2,
                                name="tmp")
                nc.vector.tensor_mul(tmp, ckbs[(at, k)],
                                     coT[:, at, k:k + CHUNK])
                nc.vector.tensor_add(accT[:, at, :], accT[:, at, :], tmp)

        # ---------------- attention ----------------------------------------
        def attention_head(h):
            at, lo = h // 2, (h % 2) * 64
            par = h % 2
            cps = [pa.tile([65, 512], dt.float32, tag=f"ctx{par}{sb}", bufs=1,
                           name=f"cps{sb}") for sb in range(2)]
            for sk in range(16):
                sc = pa.tile([128, 1024], dt.float32, tag="sc", bufs=2,
                             name="sc")
                for sb in range(2):
                    nc.tensor.matmul(
                        sc[:, sb * 512:(sb + 1) * 512],
                        kT[lo:lo + 64, at, sk * 128:(sk + 1) * 128],
                        qT[lo:lo + 64, at, sb * 512:(sb + 1) * 512],
                        start=True, stop=True)
                pt = work.tile([128, 1024], dt.bfloat16, tag="pt", bufs=3,
                               name="pt")
                nc.scalar.activation(pt, sc, Act.Exp, scale=0.125)
                for sb in range(2):
                    nc.tensor.matmul(
                        cps[sb], vsb[:, sk, h, :],
                        pt[:, sb * 512:(sb + 1) * 512],
                        start=(sk == 0), stop=(sk == 15))
            # evacuate, transpose back to [s, d] (PE), normalize (DVE)
            for sb in range(2):
                cx = work.tile([65, 512], dt.bfloat16, tag="cx", bufs=4,
                               name="cx")
                nc.vector.tensor_copy(cx, cps[sb])
                for s4 in range(4):
                    st = sb * 4 + s4
                    tp = pa.tile([128, 65], dt.bfloat16, tag="sc", bufs=2,
                                 name="tp")
                    nc.tensor.transpose(
                        tp, cx[:, s4 * 128:(s4 + 1) * 128], ident[0:65, 0:65])
                    rcp = work.tile([128, 1], dt.float32, tag="rcp", bufs=4,
                                    name="rcp")
                    nc.vector.reciprocal(rcp, tp[:, 64:65])
                    nc.vector.tensor_scalar_mul(
                        stg[:, st, h * 64:(h + 1) * 64], tp[:, 0:64], rcp)
            nc.sync.dma_start(out=out_r[:, :, h * 64:(h + 1) * 64],
                              in_=stg[:, :, h * 64:(h + 1) * 64])

        def conv_tile_out(at):
            # transpose einsum accumulators to [s, d], scale by 1/den, store
            for st in range(8):
                tp2 = pa.tile([128, 128], dt.bfloat16, tag="sc", bufs=2,
                              name="tp2")
                nc.tensor.transpose(
                    tp2, accT[:, at, st * 128:(st + 1) * 128], ident)
                for hh in range(2):
                    h = at * 2 + hh
                    nc.vector.tensor_scalar_mul(
                        stg[:, st, 384 + h * 64: 384 + (h + 1) * 64],
                        tp2[:, hh * 64:(hh + 1) * 64],
                        recipc[:, st, h:h + 1])
            for hh in range(2):
                h = at * 2 + hh
                nc.sync.dma_start(
                    out=out_r[:, :, 384 + h * 64: 384 + (h + 1) * 64],
                    in_=stg[:, :, 384 + h * 64: 384 + (h + 1) * 64])

        attention_head(0)
        attention_head(1)
        einsum_tile(0)
        conv_tile_out(0)
        attention_head(2)
        einsum_tile(1)
        attention_head(3)
        conv_tile_out(1)
        attention_head(4)
        einsum_tile(2)
        conv_tile_out(2)
        attention_head(5)

    nc.compile()
    return nc


def _prep_in_maps(inputs):
    x = np.asarray(inputs["x"], np.float32)
    dw = np.asarray(inputs["dw"], np.float32).reshape(HIDDEN, K)

    def sb_layout(wT, ntile):  # [ntile*128, F] -> [128, ntile*F]
        f = wT.shape[1]
        return np.ascontiguousarray(
            wT.reshape(ntile, 128, f).transpose(1, 0, 2).reshape(128, ntile * f))

    def wprep(w, dtype=BF16):  # [A, HIDDEN] -> [128, 6*A]
        return sb_layout(np.ascontiguousarray(w.T).astype(dtype), 6)

    com = {
        "wq": wprep(inputs["Wq"]), "wk": wprep(inputs["Wk"]),
        "wv": wprep(inputs["Wv"]), "wco": wprep(inputs["Wco"]),
        "wpw": wprep(inputs["pw"], FP16),
        "wck": sb_layout(np.ascontiguousarray(inputs["Wck"].T).astype(BF16), 3),
        "sel": np.kron(np.eye(N_HEADS), np.ones((K, 1))).astype(BF16),
        "bvrow": inputs["bv"].reshape(1, ALL_HEAD).astype(BF16),
        "bq": np.ascontiguousarray(inputs["bq"].reshape(3, 128).T, np.float32),
        "bk": np.ascontiguousarray(inputs["bk"].reshape(3, 128).T, np.float32),
        "convb": np.ascontiguousarray(
            inputs["conv_bias"].reshape(3, 128).T, np.float32),
        "bco": np.ascontiguousarray(inputs["bco"].reshape(3, 128).T, np.float32),
        "bck": inputs["bck"].reshape(54, 1).astype(np.float32),
        # dwsc[p, ct*9+k] = dw[ct*128+p, k]
        "dwsc": np.ascontiguousarray(
            dw.reshape(6, 128, K).transpose(1, 0, 2).reshape(128, 6 * K),
            np.float32),
    }

    in_maps = []
    for b in range(B):
        xb = x[b]                                   # [S, HIDDEN]
        xTb = np.ascontiguousarray(xb.T).astype(BF16)   # [768, S]
        xT_pad = np.zeros((HIDDEN, S + 8), BF16)
        xT_pad[:, 4:4 + S] = xTb
        for j in range(2):
            loc = np.ascontiguousarray(xT_pad[:, j * CHUNK: j * CHUNK + 1032])
            oth = np.ascontiguousarray(
                xTb[:, (1 - j) * CHUNK: (2 - j) * CHUNK])
            g0 = j * CHUNK - 4
            mrows = np.arange(g0, g0 + 1032)
            comask = ((mrows >= 0) & (mrows < S)).astype(BF16).reshape(1, 1032)
            m = dict(com)
            m["x_loc"] = sb_layout(loc, 6)
            m["x_oth"] = sb_layout(oth, 6)
            m["comask"] = comask
            in_maps.append(m)
    return in_maps


def _gather(results):
    # per-core out: [128, 8*768] where row s_local = st*128 + p
    outs = []
    for r in results:
        o = np.asarray(r["out"], np.float32).reshape(128, 8, 768)
        outs.append(np.ascontiguousarray(o.transpose(1, 0, 2)).reshape(1024, 768))
    full = np.stack(outs).reshape(B, 2, CHUNK, 768).reshape(B, S, 768)
    return full


def kernel(**inputs):
    from concourse.bass_utils import run_bass_kernel_spmd

    key = "prog"
    if key not in _COMPILED:
        _COMPILED[key] = _build_program()
    nc = _COMPILED[key]
    in_maps = _prep_in_maps(inputs)
    res = run_bass_kernel_spmd(nc, in_maps, list(range(N_CORES)))
    return _gather(res.results)


if __name__ == "__main__":
    import reference
    inp = {k: np.asarray(v) for k, v in reference.setup_inputs().items()}
    got = kernel(**inp)
    want = np.asarray(reference.reference(**inp))
    err = np.linalg.norm(got - want) / np.linalg.norm(want)
    print("rel err:", err)
